# revision 11
# baseline (speedup 1.0000x reference)
"""BiLSTM kernel for Trainium2 (8 NeuronCores, SPMD).

Problem: x[64,1024,256], W_f/W_b[768,4*512], b_f/b_b[2048], W_fc[1024,128],
b_fc[128] -> out[64,128] (final fwd/bwd hidden states through a Linear).

Sharding: 8 cores = 2 directions x 4 batch slices (B_loc=16 per core).
The backward direction is the forward program run on time-reversed x
(host-side flip), so all cores run one SPMD program.

Per-core design (all on-chip, no collectives):
  - W resident in SBUF as 6 k-tiles [128, 4H] (rows 0:256 = x-part, 256:768
    = h-part of the concat [x_t, h] @ W).
  - Precompute Gx = x @ W_x + b in 128-token tiles (8 timesteps x 16 batch)
    into an 8-slot SBUF ring, interleaved with the recurrence (Tile
    framework pipelines it; PE cost amortizes to ~0.3us/step).
  - Recurrence step t: gates[16,2048] accumulate in PSUM via
    (a) selector matmul E_tau.T @ gx_ring (picks the 16 batch rows of
        timestep t from the 128-token tile), then
    (b) 4x4 matmuls h.T (stationary [128,16]) x W_h (moving [128,512]).
    ACT does sigmoid(f,i)/tanh(g)/sigmoid(o)/tanh(c); DVE does c/h updates;
    PE transposes h[16,512] -> hT[128,64] for the next step's stationary.
    o-gate is emitted last so its sigmoid/h-chain overlaps other work.
  - Final: partial = h_last @ W_fc_half via hT; host adds fwd+bwd+b_fc.
"""

import sys

for _p in ("/opt/trn_rl_repo", "/root/.axon_site/_ro/trn_rl_repo"):
    if _p not in sys.path:
        sys.path.insert(0, _p)

import numpy as np

import concourse.bass as bass
import concourse.mybir as mybir
import concourse.tile as tile
from concourse import bacc

F32 = mybir.dt.float32

# Problem dims (hardcoded per harness contract)
B, T_FULL, D, H = 64, 1024, 256, 512
G4 = 4 * H  # 2048
O = 128
N_CORES = 8
B_LOC = 16  # batch rows per core
KD = D // 128  # 2 k-tiles for x-part
KH = H // 128  # 4 k-tiles for h-part
NCH = G4 // 512  # 4 psum chunks (f, i, o, g)
TPG = 8  # timesteps per precompute tile/group
RING = 8  # ring slots


def build_program(T=T_FULL):
    """Build the SPMD Bass program for one core (B_LOC batch, one direction)."""
    nc = bacc.Bacc("TRN2", target_bir_lowering=False, debug=False,
                   num_devices=N_CORES)

    x_d = nc.declare_dram_parameter("x", [B_LOC, T, D], F32, isOutput=False)
    w_d = nc.declare_dram_parameter("w", [D + H, G4], F32, isOutput=False)
    b_d = nc.declare_dram_parameter("b", [1, G4], F32, isOutput=False)
    wfc_d = nc.declare_dram_parameter("wfc", [H, O], F32, isOutput=False)
    e_d = nc.declare_dram_parameter("esel", [128, TPG * B_LOC], F32, isOutput=False)
    i_d = nc.declare_dram_parameter("ident", [128, 128], F32, isOutput=False)
    out_d = nc.declare_dram_parameter("out", [B_LOC, O], F32, isOutput=True)

    NG = T // TPG  # number of token groups

    with tile.TileContext(nc) as tc:
        with (
            tc.tile_pool(name="const", bufs=1) as cpool,
            tc.tile_pool(name="ring", bufs=RING) as rpool,
            tc.tile_pool(name="work", bufs=3) as wpool,
            tc.tile_pool(name="state", bufs=2) as spool,
            tc.tile_pool(name="psum", bufs=1, space="PSUM") as ppool,
        ):
            # ---- constants / weights in SBUF ----
            w_sb = cpool.tile([128, 6 * G4], F32)  # k-tile k at [G4*k, G4*(k+1))
            for k in range(6):
                nc.sync.dma_start(out=w_sb[:, G4 * k:G4 * (k + 1)],
                                  in_=w_d[128 * k:128 * (k + 1), :])
            wfc_sb = cpool.tile([128, KH * O], F32)
            for k in range(KH):
                nc.sync.dma_start(out=wfc_sb[:, O * k:O * (k + 1)],
                                  in_=wfc_d[128 * k:128 * (k + 1), :])
            b_sb = cpool.tile([1, G4], F32)
            nc.sync.dma_start(out=b_sb[:], in_=b_d[:])
            ones_sb = cpool.tile([1, 128], F32)
            nc.gpsimd.memset(ones_sb[:], 1.0)
            e_sb = cpool.tile([128, TPG * B_LOC], F32)
            nc.sync.dma_start(out=e_sb[:], in_=e_d[:])
            id_sb = cpool.tile([128, 128], F32)
            nc.sync.dma_start(out=id_sb[:], in_=i_d[:])

            # ---- precompute one 128-token tile of Gx into a ring slot ----
            def precompute(g):
                xt = wpool.tile([128, D], F32, tag="xt", name=f"xt{g}")
                # partitions = (b, tau): p = b*TPG + tau (row-major match)
                nc.sync.dma_start(out=xt[:], in_=x_d[:, TPG * g:TPG * (g + 1), :])
                xT_ps = ppool.tile([128, D], F32, tag="gxps", name=f"xTps{g}",
                                   bufs=1)
                for c in range(KD):
                    nc.tensor.transpose(xT_ps[:, 128 * c:128 * (c + 1)],
                                        xt[:, 128 * c:128 * (c + 1)],
                                        id_sb[:])
                xT = wpool.tile([128, D], F32, tag="xT", name=f"xT{g}")
                nc.vector.tensor_copy(xT[:], xT_ps[:])

                gx = rpool.tile([128, G4], F32, tag="gx", name=f"gx{g}")
                for half in range(2):
                    gx_ps = ppool.tile([128, 1024], F32, tag="gxps",
                                       name=f"gxps{g}_{half}", bufs=1)
                    for n in range(2):
                        sl = slice(1024 * half + 512 * n,
                                   1024 * half + 512 * n + 512)
                        psl = slice(512 * n, 512 * n + 512)
                        for k in range(KD):
                            nc.tensor.matmul(
                                gx_ps[:, psl],
                                xT[:, 128 * k:128 * (k + 1)],
                                w_sb[:, G4 * k + 1024 * half + 512 * n:
                                     G4 * k + 1024 * half + 512 * n + 512],
                                start=(k == 0), stop=False)
                        nc.tensor.matmul(gx_ps[:, psl], ones_sb[:],
                                         b_sb[:, sl], start=False, stop=True)
                    nc.vector.tensor_copy(gx[:, 1024 * half:1024 * (half + 1)],
                                          gx_ps[:])
                return gx

            gx_tiles = {}
            hT_prev = None
            c_prev = None

            # chunk order: f(0), i(1), g(3), o(2) — o last
            CH_ORDER = (0, 1, 3, 2)

            for t in range(T):
                g, tau = divmod(t, TPG)
                if tau == 0:
                    gx_tiles[g] = precompute(g)
                    if g - RING in gx_tiles:
                        del gx_tiles[g - RING]
                gx = gx_tiles[g]
                esl = e_sb[:, B_LOC * tau:B_LOC * (tau + 1)]

                gates = ppool.tile([B_LOC, G4], F32, tag="gates",
                                   name=f"gates{t}", bufs=1)
                for n in CH_ORDER:
                    sl = slice(512 * n, 512 * n + 512)
                    nc.tensor.matmul(gates[:, sl], esl, gx[:, sl],
                                     start=True, stop=(t == 0))
                    if t > 0:
                        for k in range(KH):
                            nc.tensor.matmul(
                                gates[:, sl],
                                hT_prev[:, B_LOC * k:B_LOC * (k + 1)],
                                w_sb[:, G4 * (2 + k) + 512 * n:
                                     G4 * (2 + k) + 512 * n + 512],
                                start=False, stop=(k == KH - 1))

                fi = wpool.tile([B_LOC, 2 * H], F32, tag="fi", name=f"fi{t}")
                nc.scalar.activation(fi[:], gates[:, 0:2 * H],
                                     mybir.ActivationFunctionType.Sigmoid)
                gg = wpool.tile([B_LOC, H], F32, tag="gg", name=f"gg{t}")
                nc.scalar.activation(gg[:], gates[:, 3 * H:4 * H],
                                     mybir.ActivationFunctionType.Tanh)

                c_new = spool.tile([B_LOC, H], F32, tag="c", name=f"c{t}")
                if t == 0:
                    nc.vector.tensor_mul(c_new[:], fi[:, H:2 * H], gg[:])
                else:
                    t1 = wpool.tile([B_LOC, H], F32, tag="t1", name=f"t1{t}")
                    nc.vector.tensor_mul(t1[:], fi[:, 0:H], c_prev[:])
                    t2 = wpool.tile([B_LOC, H], F32, tag="t2", name=f"t2{t}")
                    nc.vector.tensor_mul(t2[:], fi[:, H:2 * H], gg[:])
                    nc.vector.tensor_add(c_new[:], t1[:], t2[:])
                c_prev = c_new

                tc_sb = wpool.tile([B_LOC, H], F32, tag="tc", name=f"tc{t}")
                nc.scalar.activation(tc_sb[:], c_new[:],
                                     mybir.ActivationFunctionType.Tanh)
                oo = wpool.tile([B_LOC, H], F32, tag="oo", name=f"oo{t}")
                nc.scalar.activation(oo[:], gates[:, 2 * H:3 * H],
                                     mybir.ActivationFunctionType.Sigmoid)
                h_sb = wpool.tile([B_LOC, H], F32, tag="h", name=f"h{t}")
                nc.vector.tensor_mul(h_sb[:], oo[:], tc_sb[:])

                hT_ps = ppool.tile([128, KH * B_LOC], F32, tag="hT",
                                   name=f"hTps{t}", bufs=2)
                for k in range(KH):
                    nc.tensor.transpose(hT_ps[:, B_LOC * k:B_LOC * (k + 1)],
                                        h_sb[:, 128 * k:128 * (k + 1)],
                                        id_sb[0:B_LOC, 0:B_LOC])
                hT = spool.tile([128, KH * B_LOC], F32, tag="hT",
                                name=f"hT{t}")
                nc.vector.tensor_copy(hT[:], hT_ps[:])
                hT_prev = hT

            # ---- final: partial = h_last @ W_fc_half ----
            out_ps = ppool.tile([B_LOC, O], F32, tag="hT", name="outps",
                                bufs=2)
            for k in range(KH):
                nc.tensor.matmul(out_ps[:],
                                 hT_prev[:, B_LOC * k:B_LOC * (k + 1)],
                                 wfc_sb[:, O * k:O * (k + 1)],
                                 start=(k == 0), stop=(k == KH - 1))
            out_sb = wpool.tile([B_LOC, O], F32, tag="out", name="outsb")
            nc.vector.tensor_copy(out_sb[:], out_ps[:])
            nc.sync.dma_start(out=out_d[:], in_=out_sb[:])

    nc.compile()
    return nc


def _selector_np():
    """E[128, TPG*B_LOC]: E[:, tau*16:(tau+1)*16] picks rows p=b*TPG+tau
    out of a 128-token Gx tile (gates_t[b] = (E_tau.T @ gx)[b])."""
    e = np.zeros((128, TPG * B_LOC), dtype=np.float32)
    for tau in range(TPG):
        for b in range(B_LOC):
            e[b * TPG + tau, tau * B_LOC + b] = 1.0
    return e


class Runner:
    """Compile once; execute the 8-core SPMD program repeatedly.

    Mirrors bass2jax.run_bass_via_pjrt's sharded jit construction so
    repeated executions reuse one compiled NEFF with device-resident
    inputs (for steady-state timing).
    """

    def __init__(self, T=T_FULL):
        import jax
        from jax.sharding import Mesh, PartitionSpec
        from jax.experimental.shard_map import shard_map
        from concourse import bass2jax

        bass2jax.install_neuronx_cc_hook()
        self.T = T
        nc = self.nc = build_program(T)

        part_name = (nc.partition_id_tensor.name
                     if nc.partition_id_tensor else None)
        in_names, out_names, out_avals, zero_outs = [], [], [], []
        for alloc in nc.m.functions[0].allocations:
            if not isinstance(alloc, mybir.MemoryLocationSet):
                continue
            name = alloc.memorylocations[0].name
            if alloc.kind == "ExternalInput":
                if name != part_name:
                    in_names.append(name)
            elif alloc.kind == "ExternalOutput":
                out_names.append(name)
                shape = tuple(alloc.tensor_shape)
                dtype = mybir.dt.np(alloc.dtype)
                out_avals.append(jax.core.ShapedArray(shape, dtype))
                zero_outs.append(np.zeros(shape, dtype))
        self.in_names, self.out_names = in_names, out_names
        self.out_avals, self.zero_outs = out_avals, zero_outs
        n_params, n_outs = len(in_names), len(out_names)
        all_names = in_names + out_names
        if part_name is not None:
            all_names = all_names + [part_name]
        donate = tuple(range(n_params, n_params + n_outs))

        def _body(*args):
            operands = list(args)
            if part_name is not None:
                operands.append(bass2jax.partition_id_tensor())
            outs = bass2jax._bass_exec_p.bind(
                *operands,
                out_avals=tuple(out_avals),
                in_names=tuple(all_names),
                out_names=tuple(out_names),
                lowering_input_output_aliases=(),
                sim_require_finite=True,
                sim_require_nnan=True,
                nc=nc,
            )
            return tuple(outs)

        devices = jax.devices()[:N_CORES]
        self.mesh = Mesh(np.asarray(devices), ("core",))
        in_specs = (PartitionSpec("core"),) * (n_params + n_outs)
        out_specs = (PartitionSpec("core"),) * n_outs
        self.sharded = jax.jit(
            shard_map(_body, mesh=self.mesh, in_specs=in_specs,
                      out_specs=out_specs, check_rep=False),
            donate_argnums=donate, keep_unused=True)
        self._dev_in = None

    def prepare(self, in_maps):
        """Stage per-core inputs on device (concat on axis 0)."""
        import jax
        from jax.sharding import NamedSharding, PartitionSpec

        sh = NamedSharding(self.mesh, PartitionSpec("core"))
        concat_in = [
            np.concatenate([np.asarray(m[name]) for m in in_maps], axis=0)
            for name in self.in_names
        ]
        self._dev_in = [jax.device_put(a, sh) for a in concat_in]

    def exec_once(self):
        """One execution; returns per-core dict list. Blocks until done."""
        concat_zeros = [
            np.zeros((N_CORES * z.shape[0], *z.shape[1:]), z.dtype)
            for z in self.zero_outs
        ]
        out_arrs = self.sharded(*self._dev_in, *concat_zeros)
        out_arrs = [np.asarray(a) for a in out_arrs]
        return [
            {name: out_arrs[i].reshape(N_CORES, *self.out_avals[i].shape)[c]
             for i, name in enumerate(self.out_names)}
            for c in range(N_CORES)
        ]


_RUNNER_CACHE = {}


def get_runner(T=T_FULL):
    if T not in _RUNNER_CACHE:
        _RUNNER_CACHE[T] = Runner(T)
    return _RUNNER_CACHE[T]


def make_in_maps(x, W_f, b_f, W_b, b_b, W_fc):
    e_np = _selector_np()
    id_np = np.eye(128, dtype=np.float32)
    in_maps = []
    for core in range(N_CORES):
        d, j = divmod(core, 4)
        xs = x[B_LOC * j:B_LOC * (j + 1)]
        if d == 1:
            xs = xs[:, ::-1]
        in_maps.append({
            "x": np.ascontiguousarray(xs, dtype=np.float32),
            "w": np.ascontiguousarray(W_f if d == 0 else W_b, dtype=np.float32),
            "b": np.reshape(b_f if d == 0 else b_b, (1, G4)).astype(np.float32),
            "wfc": np.ascontiguousarray(
                W_fc[:H] if d == 0 else W_fc[H:], dtype=np.float32),
            "esel": e_np,
            "ident": id_np,
        })
    return in_maps


def kernel(x, W_f, b_f, W_b, b_b, W_fc, b_fc):
    x = np.asarray(x)
    runner = get_runner(x.shape[1])
    runner.prepare(make_in_maps(x, np.asarray(W_f), np.asarray(b_f),
                                np.asarray(W_b), np.asarray(b_b),
                                np.asarray(W_fc)))
    res = runner.exec_once()
    pf = np.concatenate([res[j]["out"] for j in range(4)], axis=0)
    pb = np.concatenate([res[4 + j]["out"] for j in range(4)], axis=0)
    return (pf + pb + np.asarray(b_fc)[None, :]).astype(np.float32)
